# revision 2
# baseline (speedup 1.0000x reference)
"""Trainium2 kernel for nn_EulerBias: exact Riemann-solver bias field.

Structure:
  * Host (numpy, float32): the K-interface Newton solve (tiny: B x 63) ->
    wave speeds, then per-batch coefficient matrices for the device stage.
  * Device (8 NeuronCores, batch-parallel, 2 batches/core): for every query
    point q the bias over the 64 segment columns is

        out[q,k] = min(T1[q,k],0) + min(T2[q,k],0)

    where T1/T2 are affine in (u,it,1) = (x*it, 1/(t+eps), 1) with per-k
    coefficients -> one small-contraction (Kc=12) float32r matmul on TensorE
    (1 cyc/row vs fp32's 4) produces T1||T2 for 512 queries per instruction;
    ScalarE computes relu(-T2); VectorE fuses min(T1,0) - relu(-T2) in one
    op, writing bf16; DMA stores 512KB contiguous bf16 blocks (host upcasts
    to f32 - bias absmax ~2e5, bf16 keeps rel err ~2e-3, gate is 2e-2).

Masked columns (pieces_mask == 0) are encoded in the coefficients
(T1 = -1e9, T2 = +1e30) so no separate mask pass is needed. Assumes
pieces_mask >= 0 (it is a 0/1 mask; the harness fills ones).
"""

import numpy as np

GAMMA = np.float32(1.4)
EPS = np.float32(1e-6)
N_NEWTON = 20
B, K, NT, NX = 16, 64, 128, 256
NQ = NT * NX            # 32768 queries per batch
N_CORES = 8
B_PER_CORE = B // N_CORES
# device tiling: query q = sp*4096 + m*32 + h*16 + g*4 + j
#   m: psum partition (128), h: psum-tile half (2), g: matmul-in-half (4),
#   j: contraction group (4).  One sp-iteration = 4096 queries.
CHUNK = 128
GROUPS = 4
SP_Q = 4096                      # queries per sp iteration
SP_PER_BATCH = NQ // SP_Q        # 8
BIG = np.float32(1e30)
NEGBIG = np.float32(-1e9)

_COMPILED = {}


def _f32(x):
    return np.asarray(x, dtype=np.float32)


def _host_wave_speeds(xs, ks, ks_v, ks_p):
    """Mirror of reference.py's f32 Newton solve, in numpy float32."""
    gm1 = np.float32(GAMMA - 1.0)
    gp1 = np.float32(GAMMA + 1.0)
    exp_rare = np.float32(gm1 / (2.0 * GAMMA))

    def clip_lo(v, lo=EPS):
        return np.maximum(v, lo)

    rho_L, rho_R = ks[:, :-1], ks[:, 1:]
    u_L, u_R = ks_v[:, :-1], ks_v[:, 1:]
    p_L, p_R = ks_p[:, :-1], ks_p[:, 1:]

    def sound(rho, p):
        return np.sqrt(clip_lo(GAMMA * p / clip_lo(rho)))

    c_L, c_R = sound(rho_L, p_L), sound(rho_R, p_R)
    A_L = np.float32(2.0) / (gp1 * clip_lo(rho_L))
    A_R = np.float32(2.0) / (gp1 * clip_lo(rho_R))
    B_L = gm1 / gp1 * p_L
    B_R = gm1 / gp1 * p_R

    def wave_f_df(p, p_K, A_K, B_K, c_K):
        denom = clip_lo(p + B_K)
        sqrt_AoD = np.sqrt(clip_lo(A_K / denom))
        f_shock = (p - p_K) * sqrt_AoD
        df_shock = sqrt_AoD * (np.float32(1.0) - (p - p_K) / (np.float32(2.0) * denom))
        p_ratio = clip_lo(p / clip_lo(p_K))
        f_rare = np.float32(2.0) * c_K / gm1 * (p_ratio ** exp_rare - np.float32(1.0))
        df_rare = c_K / (GAMMA * clip_lo(p_K)) * p_ratio ** np.float32(-gp1 / (2.0 * GAMMA))
        is_shock = p > p_K
        return np.where(is_shock, f_shock, f_rare), np.where(is_shock, df_shock, df_rare)

    p0 = clip_lo(((c_L + c_R - gm1 / np.float32(2.0) * (u_R - u_L))
                  / (c_L / clip_lo(p_L) ** exp_rare + c_R / clip_lo(p_R) ** exp_rare))
                 ** np.float32(1.0 / exp_rare))
    p_star = p0
    for _ in range(N_NEWTON):
        f_L, df_L = wave_f_df(p_star, p_L, A_L, B_L, c_L)
        f_R, df_R = wave_f_df(p_star, p_R, A_R, B_R, c_R)
        residual = f_L + f_R + (u_R - u_L)
        jacobian = clip_lo(df_L + df_R)
        p_star = clip_lo(p_star - residual / jacobian)

    gp1_o_2g = np.float32(gp1 / (2.0 * GAMMA))
    sigma_1 = u_L - c_L * np.sqrt(clip_lo(np.float32(1.0) + gp1_o_2g * (p_star / clip_lo(p_L) - np.float32(1.0))))
    speed_left = np.where(p_star > p_L, sigma_1, u_L - c_L)
    sigma_3 = u_R + c_R * np.sqrt(clip_lo(np.float32(1.0) + gp1_o_2g * (p_star / clip_lo(p_R) - np.float32(1.0))))
    speed_right = np.where(p_star > p_R, sigma_3, u_R + c_R)
    return speed_left.astype(np.float32), speed_right.astype(np.float32)


def _host_coef(xs, mask, sl, sr):
    """Per-batch [12, 512] coefficient matrices (the matmul's moving operand).

    psum col n = 64*j + k       (j = contraction group) -> T1
    psum col n = 256 + 64*j + k                         -> T2
    contraction rows 3j+(0,1,2) multiply (u, it, 1) of group j.
    """
    xd = xs[:, 1:K]                      # (B, 63)
    m = mask.astype(np.float32)          # (B, 64)
    act = m != 0

    # T1 = -m*u + m*xd*it + m*sr   (k < 63);  col 63 -> +BIG;  masked -> -1e9
    Wu1 = np.zeros((B, K), np.float32)
    Wi1 = np.zeros((B, K), np.float32)
    Wc1 = np.zeros((B, K), np.float32)
    Wu1[:, :63] = -m[:, :63]
    Wi1[:, :63] = m[:, :63] * xd
    Wc1[:, :63] = m[:, :63] * sr
    Wc1[:, 63] = BIG
    Wu1[~act] = 0.0
    Wi1[~act] = 0.0
    Wc1[~act] = NEGBIG

    # T2 = m*u - m*xd[k-1]*it - m*sl[k-1] (k >= 1); col 0 or masked -> +BIG
    # (so min(T2,0) = -m*relu(sl[k-1] - xi[k-1]))
    Wu2 = np.zeros((B, K), np.float32)
    Wi2 = np.zeros((B, K), np.float32)
    Wc2 = np.zeros((B, K), np.float32)
    Wu2[:, 1:] = m[:, 1:]
    Wi2[:, 1:] = -m[:, 1:] * xd
    Wc2[:, 1:] = -m[:, 1:] * sl
    Wc2[:, 0] = BIG
    Wu2[~act] = 0.0
    Wi2[~act] = 0.0
    Wc2[~act] = BIG

    coef = np.zeros((B, 3 * GROUPS, 512), np.float32)
    for j in range(GROUPS):
        c1 = slice(64 * j, 64 * j + 64)
        c2 = slice(256 + 64 * j, 256 + 64 * j + 64)
        coef[:, 3 * j + 0, c1] = Wu1
        coef[:, 3 * j + 1, c1] = Wi1
        coef[:, 3 * j + 2, c1] = Wc1
        coef[:, 3 * j + 0, c2] = Wu2
        coef[:, 3 * j + 1, c2] = Wi2
        coef[:, 3 * j + 2, c2] = Wc2
    return coef


def _host_qdata(t_coords, x_coords):
    """(B, SP, 12, 1024) stationary operands: rows 3j+(0,1,2) = (u, it, 1),
    column 512*h + 128*g + m for query q = sp*4096 + m*32 + h*16 + g*4 + j.

    The m-major query assignment makes each sp-iteration's store one
    contiguous 512KB bf16 HBM range (4KB per partition row)."""
    it = np.float32(1.0) / (t_coords.reshape(B, NQ) + EPS)
    u = x_coords.reshape(B, NQ) * it

    def lay(v):
        # (b, sp, m, h, g, j) -> [b, sp, j, (h, g, m)]
        v = v.reshape(B, SP_PER_BATCH, CHUNK, 2, GROUPS, GROUPS)
        return np.transpose(v, (0, 1, 5, 3, 4, 2)).reshape(
            B, SP_PER_BATCH, GROUPS, 2 * GROUPS * CHUNK)

    qd = np.empty((B, SP_PER_BATCH, 3 * GROUPS, 2 * GROUPS * CHUNK), np.float32)
    qd[:, :, 0::3, :] = lay(u)
    qd[:, :, 1::3, :] = lay(it)
    qd[:, :, 2::3, :] = 1.0
    return qd


def _build_nc(repeat=1):
    import concourse.bacc as bacc
    import concourse.mybir as mybir
    import concourse.tile as tile

    nc = bacc.Bacc(None, target_bir_lowering=False, debug=False)
    f32r = mybir.dt.float32r
    f32 = mybir.dt.float32
    bf16 = mybir.dt.bfloat16

    qd_d = nc.declare_dram_parameter(
        "qd", [B_PER_CORE, SP_PER_BATCH, 3 * GROUPS, 2 * GROUPS * CHUNK],
        f32r, isOutput=False)
    cf_d = nc.declare_dram_parameter(
        "cf", [B_PER_CORE, 3 * GROUPS, 512], f32r, isOutput=False)
    out_d = nc.declare_dram_parameter(
        "out", [B_PER_CORE, NQ, K], bf16, isOutput=True)

    with tile.TileContext(nc) as tc:
        with (
            tc.tile_pool(name="cf", bufs=1) as cfp,
            tc.tile_pool(name="qd", bufs=4) as qdp,
            tc.tile_pool(name="ps", bufs=2, space="PSUM") as psp,
            tc.tile_pool(name="p2", bufs=4) as p2p,
            tc.tile_pool(name="ot", bufs=4) as otp,
        ):
            cft = []
            for b in range(B_PER_CORE):
                c = cfp.tile([3 * GROUPS, 512], f32r, tag=f"cf{b}")
                nc.sync.dma_start(c[:], cf_d[b])
                cft.append(c)
            for _ in range(repeat):
                for b in range(B_PER_CORE):
                    for sp in range(SP_PER_BATCH):
                        qdt = qdp.tile([3 * GROUPS, 2 * GROUPS * CHUNK], f32r)
                        nc.sync.dma_start(qdt[:], qd_d[b, sp])
                        ot = otp.tile([128, 2, GROUPS, 256], bf16)
                        for h in range(2):
                            ps = psp.tile([128, GROUPS, 512], f32)
                            for g in range(GROUPS):
                                nc.tensor.matmul(
                                    ps[:, g, :],
                                    qdt[:, 512 * h + 128 * g:512 * h + 128 * (g + 1)],
                                    cft[b][:],
                                    start=True, stop=True,
                                )
                            p2 = p2p.tile([128, GROUPS, 256], bf16)
                            nc.scalar.activation(
                                p2[:], ps[:, :, 256:512],
                                mybir.ActivationFunctionType.Relu, scale=-1.0)
                            nc.vector.scalar_tensor_tensor(
                                out=ot[:, h],
                                in0=ps[:, :, 0:256], scalar=0.0, in1=p2[:],
                                op0=mybir.AluOpType.min,
                                op1=mybir.AluOpType.subtract)
                        q0 = sp * SP_Q
                        dst = out_d[b, q0:q0 + SP_Q, :].rearrange(
                            "(m c) k -> m (c k)", c=32)
                        src = ot[:].rearrange("m h g x -> m (h g x)")
                        nc.sync.dma_start(dst, src)
    nc.compile()
    return nc


def _get_compiled(repeat=1):
    if repeat not in _COMPILED:
        _COMPILED[repeat] = _build_nc(repeat)
    return _COMPILED[repeat]


def _prep_inputs(inputs):
    xs = _f32(inputs["xs"])
    ks = _f32(inputs["ks"])
    ks_v = _f32(inputs["ks_v"])
    ks_p = _f32(inputs["ks_p"])
    mask = _f32(inputs["pieces_mask"])
    t_coords = _f32(inputs["t_coords"])
    x_coords = _f32(inputs["x_coords"])

    sl, sr = _host_wave_speeds(xs, ks, ks_v, ks_p)
    coef = _host_coef(xs, mask, sl, sr)
    qd = _host_qdata(t_coords, x_coords)
    return [
        {
            "qd": np.ascontiguousarray(qd[c * B_PER_CORE:(c + 1) * B_PER_CORE]),
            "cf": np.ascontiguousarray(coef[c * B_PER_CORE:(c + 1) * B_PER_CORE]),
        }
        for c in range(N_CORES)
    ]


def run(inputs, trace=False):
    from concourse.bass_utils import run_bass_kernel_spmd

    in_maps = _prep_inputs(inputs)
    nc = _get_compiled()
    res = None
    for attempt in range(3):
        try:
            res = run_bass_kernel_spmd(
                nc, in_maps, core_ids=list(range(N_CORES)), trace=trace)
            break
        except Exception:
            if attempt == 2:
                raise
            import time as _time
            _time.sleep(2.0)
    out = np.empty((B, NT, NX, K), np.float32)
    for c in range(N_CORES):
        out[c * B_PER_CORE:(c + 1) * B_PER_CORE] = (
            res.results[c]["out"].astype(np.float32).reshape(B_PER_CORE, NT, NX, K))
    return out, res


def kernel(**inputs):
    out, _ = run(inputs, trace=False)
    return out


# revision 4
# speedup vs baseline: 1.5884x; 1.5884x over previous
"""Trainium2 kernel for nn_EulerBias: exact Riemann-solver bias field.

Structure:
  * Host (numpy, float32): the K-interface Newton solve (tiny: B x 63) ->
    wave speeds, then per-batch coefficient matrices for the device stage.
  * Device (8 NeuronCores, batch-parallel, 2 batches/core): for every query
    point q the bias over the 64 segment columns is

        out[q,k] = min(T1[q,k],0) + min(T2[q,k],0)

    where T1/T2 are affine in (u,it,1) = (x*it, 1/(t+eps), 1) with per-k
    coefficients -> one small-contraction (Kc=12) float32r matmul on TensorE
    (1 cyc/row vs fp32's 4) produces T1||T2 for 512 queries per instruction;
    ScalarE computes relu(-T2); VectorE fuses min(T1,0) - relu(-T2) in one
    op, writing bf16; DMA stores 512KB contiguous bf16 blocks (host upcasts
    to f32 - bias absmax ~2e5, bf16 keeps rel err ~2e-3, gate is 2e-2).

Masked columns (pieces_mask == 0) are encoded in the coefficients
(T1 = -1e9, T2 = +1e30) so no separate mask pass is needed. Assumes
pieces_mask >= 0 (it is a 0/1 mask; the harness fills ones).
"""

import numpy as np

GAMMA = np.float32(1.4)
EPS = np.float32(1e-6)
N_NEWTON = 20
B, K, NT, NX = 16, 64, 128, 256
NQ = NT * NX            # 32768 queries per batch
N_CORES = 8
B_PER_CORE = B // N_CORES
# device tiling: query q = sp*4096 + m*32 + h*16 + g*4 + j
#   m: psum partition (128), h: psum-tile half (2), g: matmul-in-half (4),
#   j: contraction group (4).  One sp-iteration = 4096 queries.
CHUNK = 128
GROUPS = 4
SP_Q = 4096                      # queries per sp iteration
SP_PER_BATCH = NQ // SP_Q        # 8
BIG = np.float32(1e30)
NEGBIG = np.float32(-1e9)

_COMPILED = {}


def _f32(x):
    return np.asarray(x, dtype=np.float32)


def _host_wave_speeds(xs, ks, ks_v, ks_p):
    """Mirror of reference.py's f32 Newton solve, in numpy float32."""
    gm1 = np.float32(GAMMA - 1.0)
    gp1 = np.float32(GAMMA + 1.0)
    exp_rare = np.float32(gm1 / (2.0 * GAMMA))

    def clip_lo(v, lo=EPS):
        return np.maximum(v, lo)

    rho_L, rho_R = ks[:, :-1], ks[:, 1:]
    u_L, u_R = ks_v[:, :-1], ks_v[:, 1:]
    p_L, p_R = ks_p[:, :-1], ks_p[:, 1:]

    def sound(rho, p):
        return np.sqrt(clip_lo(GAMMA * p / clip_lo(rho)))

    c_L, c_R = sound(rho_L, p_L), sound(rho_R, p_R)
    A_L = np.float32(2.0) / (gp1 * clip_lo(rho_L))
    A_R = np.float32(2.0) / (gp1 * clip_lo(rho_R))
    B_L = gm1 / gp1 * p_L
    B_R = gm1 / gp1 * p_R

    def wave_f_df(p, p_K, A_K, B_K, c_K):
        denom = clip_lo(p + B_K)
        sqrt_AoD = np.sqrt(clip_lo(A_K / denom))
        f_shock = (p - p_K) * sqrt_AoD
        df_shock = sqrt_AoD * (np.float32(1.0) - (p - p_K) / (np.float32(2.0) * denom))
        p_ratio = clip_lo(p / clip_lo(p_K))
        f_rare = np.float32(2.0) * c_K / gm1 * (p_ratio ** exp_rare - np.float32(1.0))
        df_rare = c_K / (GAMMA * clip_lo(p_K)) * p_ratio ** np.float32(-gp1 / (2.0 * GAMMA))
        is_shock = p > p_K
        return np.where(is_shock, f_shock, f_rare), np.where(is_shock, df_shock, df_rare)

    p0 = clip_lo(((c_L + c_R - gm1 / np.float32(2.0) * (u_R - u_L))
                  / (c_L / clip_lo(p_L) ** exp_rare + c_R / clip_lo(p_R) ** exp_rare))
                 ** np.float32(1.0 / exp_rare))
    p_star = p0
    for _ in range(N_NEWTON):
        f_L, df_L = wave_f_df(p_star, p_L, A_L, B_L, c_L)
        f_R, df_R = wave_f_df(p_star, p_R, A_R, B_R, c_R)
        residual = f_L + f_R + (u_R - u_L)
        jacobian = clip_lo(df_L + df_R)
        p_star = clip_lo(p_star - residual / jacobian)

    gp1_o_2g = np.float32(gp1 / (2.0 * GAMMA))
    sigma_1 = u_L - c_L * np.sqrt(clip_lo(np.float32(1.0) + gp1_o_2g * (p_star / clip_lo(p_L) - np.float32(1.0))))
    speed_left = np.where(p_star > p_L, sigma_1, u_L - c_L)
    sigma_3 = u_R + c_R * np.sqrt(clip_lo(np.float32(1.0) + gp1_o_2g * (p_star / clip_lo(p_R) - np.float32(1.0))))
    speed_right = np.where(p_star > p_R, sigma_3, u_R + c_R)
    return speed_left.astype(np.float32), speed_right.astype(np.float32)


def _host_coef(xs, mask, sl, sr):
    """Per-batch [12, 512] coefficient matrices (the matmul's moving operand).

    psum col n = 64*j + k       (j = contraction group) -> T1
    psum col n = 256 + 64*j + k                         -> T2
    contraction rows 3j+(0,1,2) multiply (u, it, 1) of group j.
    """
    xd = xs[:, 1:K]                      # (B, 63)
    m = mask.astype(np.float32)          # (B, 64)
    act = m != 0

    # T1 = -m*u + m*xd*it + m*sr   (k < 63);  col 63 -> +BIG;  masked -> -1e9
    Wu1 = np.zeros((B, K), np.float32)
    Wi1 = np.zeros((B, K), np.float32)
    Wc1 = np.zeros((B, K), np.float32)
    Wu1[:, :63] = -m[:, :63]
    Wi1[:, :63] = m[:, :63] * xd
    Wc1[:, :63] = m[:, :63] * sr
    Wc1[:, 63] = BIG
    Wu1[~act] = 0.0
    Wi1[~act] = 0.0
    Wc1[~act] = NEGBIG

    # T2 = m*u - m*xd[k-1]*it - m*sl[k-1] (k >= 1); col 0 or masked -> +BIG
    # (so min(T2,0) = -m*relu(sl[k-1] - xi[k-1]))
    Wu2 = np.zeros((B, K), np.float32)
    Wi2 = np.zeros((B, K), np.float32)
    Wc2 = np.zeros((B, K), np.float32)
    Wu2[:, 1:] = m[:, 1:]
    Wi2[:, 1:] = -m[:, 1:] * xd
    Wc2[:, 1:] = -m[:, 1:] * sl
    Wc2[:, 0] = BIG
    Wu2[~act] = 0.0
    Wi2[~act] = 0.0
    Wc2[~act] = BIG

    coef = np.zeros((B, 3 * GROUPS, 512), np.float32)
    for j in range(GROUPS):
        c1 = slice(64 * j, 64 * j + 64)
        c2 = slice(256 + 64 * j, 256 + 64 * j + 64)
        coef[:, 3 * j + 0, c1] = Wu1
        coef[:, 3 * j + 1, c1] = Wi1
        coef[:, 3 * j + 2, c1] = Wc1
        coef[:, 3 * j + 0, c2] = Wu2
        coef[:, 3 * j + 1, c2] = Wi2
        coef[:, 3 * j + 2, c2] = Wc2
    return coef


def _host_qdata(t_coords, x_coords):
    """(B, SP, 12, 1024) stationary operands: rows 3j+(0,1,2) = (u, it, 1),
    column 512*h + 128*g + m for query q = sp*4096 + m*32 + h*16 + g*4 + j.

    The m-major query assignment makes each sp-iteration's store one
    contiguous 512KB bf16 HBM range (4KB per partition row)."""
    it = np.float32(1.0) / (t_coords.reshape(B, NQ) + EPS)
    u = x_coords.reshape(B, NQ) * it

    def lay(v):
        # (b, sp, m, h, g, j) -> [b, sp, j, (h, g, m)]
        v = v.reshape(B, SP_PER_BATCH, CHUNK, 2, GROUPS, GROUPS)
        return np.transpose(v, (0, 1, 5, 3, 4, 2)).reshape(
            B, SP_PER_BATCH, GROUPS, 2 * GROUPS * CHUNK)

    qd = np.empty((B, SP_PER_BATCH, 3 * GROUPS, 2 * GROUPS * CHUNK), np.float32)
    qd[:, :, 0::3, :] = lay(u)
    qd[:, :, 1::3, :] = lay(it)
    qd[:, :, 2::3, :] = 1.0
    return qd


def _build_nc(repeat=1):
    import concourse.bacc as bacc
    import concourse.mybir as mybir
    import concourse.tile as tile

    nc = bacc.Bacc(None, target_bir_lowering=False, debug=False)
    f32r = mybir.dt.float32r
    f32 = mybir.dt.float32
    bf16 = mybir.dt.bfloat16

    qd_d = nc.declare_dram_parameter(
        "qd", [B_PER_CORE, SP_PER_BATCH, 3 * GROUPS, 2 * GROUPS * CHUNK],
        f32r, isOutput=False)
    cf_d = nc.declare_dram_parameter(
        "cf", [B_PER_CORE, 3 * GROUPS, 512], f32r, isOutput=False)
    out_d = nc.declare_dram_parameter(
        "out", [B_PER_CORE, NQ, K], bf16, isOutput=True)

    with tile.TileContext(nc) as tc:
        with (
            tc.tile_pool(name="cf", bufs=1) as cfp,
            tc.tile_pool(name="qd", bufs=4) as qdp,
            tc.tile_pool(name="ps", bufs=2, space="PSUM") as psp,
            tc.tile_pool(name="p2", bufs=4) as p2p,
            tc.tile_pool(name="ot", bufs=4) as otp,
        ):
            cft = []
            for b in range(B_PER_CORE):
                c = cfp.tile([3 * GROUPS, 512], f32r, tag=f"cf{b}")
                # loads ride SWDGE (GpSimd) so they never queue behind the
                # 512KB output stores on the HWDGE rings
                nc.gpsimd.dma_start(c[:], cf_d[b])
                cft.append(c)
            for _ in range(repeat):
                for b in range(B_PER_CORE):
                    for sp in range(SP_PER_BATCH):
                        qdt = qdp.tile([3 * GROUPS, 2 * GROUPS * CHUNK], f32r)
                        nc.gpsimd.dma_start(qdt[:], qd_d[b, sp])
                        ot = otp.tile([128, 2, GROUPS, 256], bf16)
                        for h in range(2):
                            ps = psp.tile([128, GROUPS, 512], f32)
                            for g in range(GROUPS):
                                nc.tensor.matmul(
                                    ps[:, g, :],
                                    qdt[:, 512 * h + 128 * g:512 * h + 128 * (g + 1)],
                                    cft[b][:],
                                    start=True, stop=True,
                                )
                            p2 = p2p.tile([128, GROUPS, 256], bf16)
                            nc.scalar.activation(
                                p2[:], ps[:, :, 256:512],
                                mybir.ActivationFunctionType.Relu, scale=-1.0)
                            nc.vector.scalar_tensor_tensor(
                                out=ot[:, h],
                                in0=ps[:, :, 0:256], scalar=0.0, in1=p2[:],
                                op0=mybir.AluOpType.min,
                                op1=mybir.AluOpType.subtract)
                        q0 = sp * SP_Q
                        dst = out_d[b, q0:q0 + SP_Q, :].rearrange(
                            "(m c) k -> m (c k)", c=32)
                        src = ot[:].rearrange("m h g x -> m (h g x)")
                        # alternate stores across the two HWDGE rings
                        # (SP / ACT) so consecutive stores overlap
                        eng = nc.sync if (b * SP_PER_BATCH + sp) % 2 == 0 else nc.scalar
                        eng.dma_start(dst, src)
    nc.compile()
    return nc


def _get_compiled(repeat=1):
    if repeat not in _COMPILED:
        _COMPILED[repeat] = _build_nc(repeat)
    return _COMPILED[repeat]


def _prep_inputs(inputs):
    xs = _f32(inputs["xs"])
    ks = _f32(inputs["ks"])
    ks_v = _f32(inputs["ks_v"])
    ks_p = _f32(inputs["ks_p"])
    mask = _f32(inputs["pieces_mask"])
    t_coords = _f32(inputs["t_coords"])
    x_coords = _f32(inputs["x_coords"])

    sl, sr = _host_wave_speeds(xs, ks, ks_v, ks_p)
    coef = _host_coef(xs, mask, sl, sr)
    qd = _host_qdata(t_coords, x_coords)
    return [
        {
            "qd": np.ascontiguousarray(qd[c * B_PER_CORE:(c + 1) * B_PER_CORE]),
            "cf": np.ascontiguousarray(coef[c * B_PER_CORE:(c + 1) * B_PER_CORE]),
        }
        for c in range(N_CORES)
    ]


def run(inputs, trace=False):
    from concourse.bass_utils import run_bass_kernel_spmd

    in_maps = _prep_inputs(inputs)
    nc = _get_compiled()
    res = None
    for attempt in range(3):
        try:
            res = run_bass_kernel_spmd(
                nc, in_maps, core_ids=list(range(N_CORES)), trace=trace)
            break
        except Exception:
            if attempt == 2:
                raise
            import time as _time
            _time.sleep(2.0)
    out = np.empty((B, NT, NX, K), np.float32)
    for c in range(N_CORES):
        out[c * B_PER_CORE:(c + 1) * B_PER_CORE] = (
            res.results[c]["out"].astype(np.float32).reshape(B_PER_CORE, NT, NX, K))
    return out, res


def kernel(**inputs):
    out, _ = run(inputs, trace=False)
    return out


# revision 8
# speedup vs baseline: 2.8412x; 1.7887x over previous
"""Trainium2 kernel for nn_EulerBias: exact Riemann-solver bias field.

Structure:
  * Host (numpy, float32): the K-interface Newton solve (tiny: B x 63) ->
    wave speeds, then per-batch coefficient matrices for the device stage.
  * Device (8 NeuronCores, batch-parallel, 2 batches/core): for every query
    point q the bias over the 64 segment columns is

        out[q,k] = min(T1[q,k],0) + min(T2[q,k],0)

    where T1/T2 are affine in (u,it,1) = (x*it, 1/(t+eps), 1) with per-k
    coefficients -> one small-contraction (Kc=12) float32r matmul on TensorE
    (1 cyc/row vs fp32's 4) produces T1||T2 for 512 queries per instruction;
    ScalarE computes relu(-T2); VectorE fuses min(T1,0) - relu(-T2) in one
    op, writing bf16; DMA stores 512KB contiguous bf16 blocks (host upcasts
    to f32 - bias absmax ~2e5, bf16 keeps rel err ~2e-3, gate is 2e-2).

Masked columns (pieces_mask == 0) are encoded in the coefficients
(T1 = -1e9, T2 = +1e30) so no separate mask pass is needed. Assumes
pieces_mask >= 0 (it is a 0/1 mask; the harness fills ones).
"""

import numpy as np

GAMMA = np.float32(1.4)
EPS = np.float32(1e-6)
N_NEWTON = 20
B, K, NT, NX = 16, 64, 128, 256
NQ = NT * NX            # 32768 queries per batch
N_CORES = 8
B_PER_CORE = B // N_CORES
# device tiling: query q = sp*4096 + m*32 + h*16 + g*4 + j
#   m: psum partition (128), h: psum-tile half (2), g: matmul-in-half (4),
#   j: contraction group (4).  One sp-iteration = 4096 queries.
CHUNK = 128
GROUPS = 4
SP_Q = 4096                      # queries per sp iteration
SP_PER_BATCH = NQ // SP_Q        # 8
BIG = np.float32(1e30)
NEGBIG = np.float32(-1e9)

_COMPILED = {}


def _f32(x):
    return np.asarray(x, dtype=np.float32)


def _host_wave_speeds(xs, ks, ks_v, ks_p):
    """Mirror of reference.py's f32 Newton solve, in numpy float32."""
    gm1 = np.float32(GAMMA - 1.0)
    gp1 = np.float32(GAMMA + 1.0)
    exp_rare = np.float32(gm1 / (2.0 * GAMMA))

    def clip_lo(v, lo=EPS):
        return np.maximum(v, lo)

    rho_L, rho_R = ks[:, :-1], ks[:, 1:]
    u_L, u_R = ks_v[:, :-1], ks_v[:, 1:]
    p_L, p_R = ks_p[:, :-1], ks_p[:, 1:]

    def sound(rho, p):
        return np.sqrt(clip_lo(GAMMA * p / clip_lo(rho)))

    c_L, c_R = sound(rho_L, p_L), sound(rho_R, p_R)
    A_L = np.float32(2.0) / (gp1 * clip_lo(rho_L))
    A_R = np.float32(2.0) / (gp1 * clip_lo(rho_R))
    B_L = gm1 / gp1 * p_L
    B_R = gm1 / gp1 * p_R

    def wave_f_df(p, p_K, A_K, B_K, c_K):
        denom = clip_lo(p + B_K)
        sqrt_AoD = np.sqrt(clip_lo(A_K / denom))
        f_shock = (p - p_K) * sqrt_AoD
        df_shock = sqrt_AoD * (np.float32(1.0) - (p - p_K) / (np.float32(2.0) * denom))
        p_ratio = clip_lo(p / clip_lo(p_K))
        f_rare = np.float32(2.0) * c_K / gm1 * (p_ratio ** exp_rare - np.float32(1.0))
        df_rare = c_K / (GAMMA * clip_lo(p_K)) * p_ratio ** np.float32(-gp1 / (2.0 * GAMMA))
        is_shock = p > p_K
        return np.where(is_shock, f_shock, f_rare), np.where(is_shock, df_shock, df_rare)

    p0 = clip_lo(((c_L + c_R - gm1 / np.float32(2.0) * (u_R - u_L))
                  / (c_L / clip_lo(p_L) ** exp_rare + c_R / clip_lo(p_R) ** exp_rare))
                 ** np.float32(1.0 / exp_rare))
    p_star = p0
    for _ in range(N_NEWTON):
        f_L, df_L = wave_f_df(p_star, p_L, A_L, B_L, c_L)
        f_R, df_R = wave_f_df(p_star, p_R, A_R, B_R, c_R)
        residual = f_L + f_R + (u_R - u_L)
        jacobian = clip_lo(df_L + df_R)
        p_star = clip_lo(p_star - residual / jacobian)

    gp1_o_2g = np.float32(gp1 / (2.0 * GAMMA))
    sigma_1 = u_L - c_L * np.sqrt(clip_lo(np.float32(1.0) + gp1_o_2g * (p_star / clip_lo(p_L) - np.float32(1.0))))
    speed_left = np.where(p_star > p_L, sigma_1, u_L - c_L)
    sigma_3 = u_R + c_R * np.sqrt(clip_lo(np.float32(1.0) + gp1_o_2g * (p_star / clip_lo(p_R) - np.float32(1.0))))
    speed_right = np.where(p_star > p_R, sigma_3, u_R + c_R)
    return speed_left.astype(np.float32), speed_right.astype(np.float32)


def _host_coef(xs, mask, sl, sr):
    """Per-batch [12, 512] coefficient matrices (the matmul's moving operand).

    psum col n = 64*j + k       (j = contraction group) -> T1
    psum col n = 256 + 64*j + k                         -> T2
    contraction rows 3j+(0,1,2) multiply (u, it, 1) of group j.
    """
    xd = xs[:, 1:K]                      # (B, 63)
    m = mask.astype(np.float32)          # (B, 64)
    act = m != 0

    # T1 = -m*u + m*xd*it + m*sr   (k < 63);  col 63 -> +BIG;  masked -> -1e9
    Wu1 = np.zeros((B, K), np.float32)
    Wi1 = np.zeros((B, K), np.float32)
    Wc1 = np.zeros((B, K), np.float32)
    Wu1[:, :63] = -m[:, :63]
    Wi1[:, :63] = m[:, :63] * xd
    Wc1[:, :63] = m[:, :63] * sr
    Wc1[:, 63] = BIG
    Wu1[~act] = 0.0
    Wi1[~act] = 0.0
    Wc1[~act] = NEGBIG

    # T2 = m*u - m*xd[k-1]*it - m*sl[k-1] (k >= 1); col 0 or masked -> +BIG
    # (so min(T2,0) = -m*relu(sl[k-1] - xi[k-1]))
    Wu2 = np.zeros((B, K), np.float32)
    Wi2 = np.zeros((B, K), np.float32)
    Wc2 = np.zeros((B, K), np.float32)
    Wu2[:, 1:] = m[:, 1:]
    Wi2[:, 1:] = -m[:, 1:] * xd
    Wc2[:, 1:] = -m[:, 1:] * sl
    Wc2[:, 0] = BIG
    Wu2[~act] = 0.0
    Wi2[~act] = 0.0
    Wc2[~act] = BIG

    coef = np.zeros((B, 3 * GROUPS, 512), np.float32)
    for j in range(GROUPS):
        c1 = slice(64 * j, 64 * j + 64)
        c2 = slice(256 + 64 * j, 256 + 64 * j + 64)
        coef[:, 3 * j + 0, c1] = Wu1
        coef[:, 3 * j + 1, c1] = Wi1
        coef[:, 3 * j + 2, c1] = Wc1
        coef[:, 3 * j + 0, c2] = Wu2
        coef[:, 3 * j + 1, c2] = Wi2
        coef[:, 3 * j + 2, c2] = Wc2
    return coef


def _host_qdata(t_coords, x_coords):
    """(B, SP, 12, 1024) stationary operands: rows 3j+(0,1,2) = (u, it, 1),
    column 512*h + 128*g + m for query q = sp*4096 + m*32 + h*16 + g*4 + j.

    The m-major query assignment makes each sp-iteration's store one
    contiguous 512KB bf16 HBM range (4KB per partition row)."""
    it = np.float32(1.0) / (t_coords.reshape(B, NQ) + EPS)
    u = x_coords.reshape(B, NQ) * it

    def lay(v):
        # (b, sp, m, h, g, j) -> [b, sp, j, (h, g, m)]
        v = v.reshape(B, SP_PER_BATCH, CHUNK, 2, GROUPS, GROUPS)
        return np.transpose(v, (0, 1, 5, 3, 4, 2)).reshape(
            B, SP_PER_BATCH, GROUPS, 2 * GROUPS * CHUNK)

    qd = np.empty((B, SP_PER_BATCH, 3 * GROUPS, 2 * GROUPS * CHUNK), np.float32)
    qd[:, :, 0::3, :] = lay(u)
    qd[:, :, 1::3, :] = lay(it)
    qd[:, :, 2::3, :] = 1.0
    return qd


def _build_nc(repeat=1):
    import concourse.bacc as bacc
    import concourse.mybir as mybir
    import concourse.tile as tile

    nc = bacc.Bacc(None, target_bir_lowering=False, debug=False)
    f32r = mybir.dt.float32r
    f32 = mybir.dt.float32
    bf16 = mybir.dt.bfloat16

    qd_d = nc.declare_dram_parameter(
        "qd", [B_PER_CORE, SP_PER_BATCH, 3 * GROUPS, 2 * GROUPS * CHUNK],
        f32r, isOutput=False)
    cf_d = nc.declare_dram_parameter(
        "cf", [B_PER_CORE, 3 * GROUPS, 512], f32r, isOutput=False)
    out_d = nc.declare_dram_parameter(
        "out", [B_PER_CORE, NQ, K], bf16, isOutput=True)

    with tile.TileContext(nc) as tc:
        with (
            tc.tile_pool(name="cf", bufs=1) as cfp,
            tc.tile_pool(name="qd", bufs=6) as qdp,
            tc.tile_pool(name="ps", bufs=4, space="PSUM") as psp,
            tc.tile_pool(name="p2", bufs=8) as p2p,
            tc.tile_pool(name="ot", bufs=6) as otp,
        ):
            cft = []
            for b in range(B_PER_CORE):
                c = cfp.tile([3 * GROUPS, 512], f32r, tag=f"cf{b}")
                # cf via HWDGE: at program start the store ring is empty, and
                # HWDGE's ~600ns first-byte beats SWDGE's serialized ~1us
                # descriptor generation - faster first matmul
                nc.sync.dma_start(c[:], cf_d[b])
                cft.append(c)
            for _ in range(repeat):
                for b in range(B_PER_CORE):
                    for sp in range(SP_PER_BATCH):
                        qdt = qdp.tile([3 * GROUPS, 2 * GROUPS * CHUNK], f32r)
                        nc.gpsimd.dma_start(qdt[:], qd_d[b, sp])
                        ot = otp.tile([128, 2, GROUPS, 256], bf16)
                        # 2-bank psum tiles, 4 in rotation: the psum-reuse
                        # dependency cycle DVE(t) -> MM(t+4) -> ACT -> DVE
                        # amortizes over 4 tile-slots instead of 2
                        for h in range(2):
                            for gp in range(2):
                                ps = psp.tile([128, 2, 512], f32)
                                for g2 in range(2):
                                    g = 2 * gp + g2
                                    nc.tensor.matmul(
                                        ps[:, g2, :],
                                        qdt[:, 512 * h + 128 * g:512 * h + 128 * (g + 1)],
                                        cft[b][:],
                                        start=True, stop=True,
                                    )
                                p2 = p2p.tile([128, 2, 256], bf16)
                                nc.scalar.activation(
                                    p2[:], ps[:, :, 256:512],
                                    mybir.ActivationFunctionType.Relu, scale=-1.0)
                                nc.vector.scalar_tensor_tensor(
                                    out=ot[:, h, 2 * gp:2 * gp + 2],
                                    in0=ps[:, :, 0:256], scalar=0.0, in1=p2[:],
                                    op0=mybir.AluOpType.min,
                                    op1=mybir.AluOpType.subtract)
                        q0 = sp * SP_Q
                        dst = out_d[b, q0:q0 + SP_Q, :].rearrange(
                            "(m c) k -> m (c k)", c=32)
                        src = ot[:].rearrange("m h g x -> m (h g x)")
                        nc.sync.dma_start(dst, src)
    nc.compile()
    return nc


def _get_compiled(repeat=1):
    if repeat not in _COMPILED:
        _COMPILED[repeat] = _build_nc(repeat)
    return _COMPILED[repeat]


def _prep_inputs(inputs):
    xs = _f32(inputs["xs"])
    ks = _f32(inputs["ks"])
    ks_v = _f32(inputs["ks_v"])
    ks_p = _f32(inputs["ks_p"])
    mask = _f32(inputs["pieces_mask"])
    t_coords = _f32(inputs["t_coords"])
    x_coords = _f32(inputs["x_coords"])

    sl, sr = _host_wave_speeds(xs, ks, ks_v, ks_p)
    coef = _host_coef(xs, mask, sl, sr)
    qd = _host_qdata(t_coords, x_coords)
    return [
        {
            "qd": np.ascontiguousarray(qd[c * B_PER_CORE:(c + 1) * B_PER_CORE]),
            "cf": np.ascontiguousarray(coef[c * B_PER_CORE:(c + 1) * B_PER_CORE]),
        }
        for c in range(N_CORES)
    ]


def run(inputs, trace=False):
    from concourse.bass_utils import run_bass_kernel_spmd

    in_maps = _prep_inputs(inputs)
    nc = _get_compiled()
    res = None
    for attempt in range(3):
        try:
            res = run_bass_kernel_spmd(
                nc, in_maps, core_ids=list(range(N_CORES)), trace=trace)
            break
        except Exception:
            if attempt == 2:
                raise
            import time as _time
            _time.sleep(2.0)
    out = np.empty((B, NT, NX, K), np.float32)
    for c in range(N_CORES):
        out[c * B_PER_CORE:(c + 1) * B_PER_CORE] = (
            res.results[c]["out"].astype(np.float32).reshape(B_PER_CORE, NT, NX, K))
    return out, res


def kernel(**inputs):
    out, _ = run(inputs, trace=False)
    return out
